# revision 5
# baseline (speedup 1.0000x reference)
"""Trainium2 Bass kernel for nn_Bert_BiLSTM (segment-mean pooling + BiLSTM).

Strategy (8 NeuronCores, data-parallel over batch, Bc=8 per core):
  Phase A (pooling): pooledT[d,w] = hidden[t,d]^T @ M_scaled[t,w] via fp32r
      matmuls, where M_scaled is the host-built one-hot(word_ids)/count
      matrix (index preprocessing only).
  Phase B (projection): pre[g,w] = w_ih^T @ pooledT (fp32r) + bias, stored
      bf16. Quarter (0,q0)+(1,q3) computed up front; the remaining items are
      dripped one gate-chunk at a time into the scan's PE idle windows.
  Phase C (scan): 256 sequential LSTM steps per direction, both directions
      interleaved anti-phase on each core. Gates in [G-part, B-free] layout,
      ONE psum bank per step-dir: identity matmul injects pre_t (opens the
      accumulation group), 16 w_hh matmuls accumulate on top. All four gates
      go through ONE sigmoid ACT (g-gate weights are pre-scaled x2 on host;
      tanh(x) = 2*sigmoid(2x)-1 fixed up with a fused tensor_scalar on
      GpSimd). c update on DVE, tanh(c) on ACT, h write as one strided DVE
      mul.
  Phase D: PE-transpose h history to [w, h] layout and DMA out; first half
      dripped through the last scan block.

Host side: shard batch, build M_scaled, permute gates to [i,f,o,g] order,
scale g columns x2, cast weights, assemble/concat outputs.
"""

import os
import sys

for _p in ("/opt/trn_rl_repo", "/root/.axon_site/_ro/trn_rl_repo"):
    if os.path.isdir(_p) and _p not in sys.path:
        sys.path.append(_p)

import numpy as np
import ml_dtypes

NCORES = 8
BC = 8          # batch per core
T = 512
D = 768
W = 256
H = 256
G = 1024        # 4*H
NT = T // 128   # 4 t-tiles
ND = D // 128   # 6 d-chunks
NG = G // 128   # 8 gate chunks (per direction)
KT = H // 128   # 2 h-chunks

_NC_CACHE = {}


def build_nc():
    """Build and compile the per-core Bass program (SPMD, same on all cores)."""
    import concourse.bacc as bacc
    import concourse.tile as tile
    from concourse import mybir
    from concourse.masks import make_identity

    f32 = mybir.dt.float32
    f32r = mybir.dt.float32r
    bf16 = mybir.dt.bfloat16
    AF = mybir.ActivationFunctionType
    ALU = mybir.AluOpType

    nc = bacc.Bacc("TRN2", target_bir_lowering=False, debug=False,
                   enable_asserts=False, num_devices=NCORES)

    hs = nc.dram_tensor("hs", [BC, NT, 128, D], f32r, kind="ExternalInput")
    msc = nc.dram_tensor("msc", [BC, NT, 128, W], f32r, kind="ExternalInput")
    wih = nc.dram_tensor("wih", [2, ND, 128, G], f32r, kind="ExternalInput")
    whh = nc.dram_tensor("whh", [2, KT, 128, G], bf16, kind="ExternalInput")
    bias = nc.dram_tensor("bias", [2 * NG, 128], f32, kind="ExternalInput")
    outf = nc.dram_tensor("outf", [BC, W, H], f32, kind="ExternalOutput")
    outb = nc.dram_tensor("outb", [BC, W, H], f32, kind="ExternalOutput")

    with tile.TileContext(nc) as tc:
        from contextlib import ExitStack
        ctx = ExitStack()
        with ctx:
            const = ctx.enter_context(tc.tile_pool(name="const", bufs=1))
            whh_sb = const.tile([128, 2, KT, G], bf16)
            nc.sync.dma_start(out=whh_sb, in_=whh.ap().rearrange("d k p g -> p d k g"))
            bias_sb = const.tile([128, 2 * NG], f32)
            nc.sync.dma_start(out=bias_sb, in_=bias.ap().rearrange("n p -> p n"))
            ident = const.tile([128, 128], bf16)
            make_identity(nc, ident)
            ident_pre = const.tile([128, 128], bf16)
            make_identity(nc, ident_pre)

            pooledT = const.tile([128, BC, ND, W], f32r)    # 48KB/part
            pre = const.tile([128, 2, W, NG, BC], bf16)     # 64KB/part
            hh = const.tile([128, 2, KT, BC, W + 1], bf16)  # h history
            cc = const.tile([128, 2, KT, BC], f32)

            # ---- Phase A: pooling ----
            with tc.tile_pool(name="hsst", bufs=3) as hsp, \
                 tc.tile_pool(name="mscst", bufs=2) as mscp, \
                 tc.tile_pool(name="psA", bufs=6, space="PSUM") as psA:
                for b in range(BC):
                    hst = []
                    msct = []
                    for tt in range(NT):
                        ht = hsp.tile([128, D], f32r, tag=f"hs{tt}")
                        nc.sync.dma_start(out=ht, in_=hs.ap()[b, tt])
                        hst.append(ht)
                        mt = mscp.tile([128, W], f32r, tag=f"ms{tt}")
                        nc.sync.dma_start(out=mt, in_=msc.ap()[b, tt])
                        msct.append(mt)
                    for dc in range(ND):
                        pps = psA.tile([128, W], f32)
                        for tt in range(NT):
                            nc.tensor.matmul(
                                out=pps,
                                lhsT=hst[tt][:, dc * 128:(dc + 1) * 128],
                                rhs=msct[tt],
                                start=(tt == 0), stop=(tt == NT - 1))
                        if (b * ND + dc) % 2 == 0:
                            nc.scalar.copy(pooledT[:, b, dc, :], pps)
                        else:
                            nc.vector.tensor_copy(pooledT[:, b, dc, :], pps)

            # scan pools first so the proj/psD pool stacks can close in
            # LIFO order around them
            bc_ctx = ctx.enter_context(ExitStack())
            psC = bc_ctx.enter_context(tc.tile_pool(name="psC", bufs=3, space="PSUM"))
            sp = bc_ctx.enter_context(tc.tile_pool(name="sp", bufs=3))
            gp = bc_ctx.enter_context(tc.tile_pool(name="gp", bufs=3))
            tp = bc_ctx.enter_context(tc.tile_pool(name="tp", bufs=3))
            thp = bc_ctx.enter_context(tc.tile_pool(name="thp", bufs=3))

            # ---- Phase B: projection; (0,q0)+(1,q3) up front, rest dripped ----
            pb_ctx = ExitStack()
            wihp = pb_ctx.enter_context(tc.tile_pool(name="wihp", bufs=1))
            psB = pb_ctx.enter_context(tc.tile_pool(name="psB", bufs=2, space="PSUM"))
            wih_f = wihp.tile([128, ND, G], f32r, tag="wf")
            nc.sync.dma_start(out=wih_f, in_=wih.ap()[0].rearrange("c p g -> p c g"))
            wih_b = wihp.tile([128, ND, G], f32r, tag="wb")
            nc.sync.dma_start(out=wih_b, in_=wih.ap()[1].rearrange("c p g -> p c g"))

            def proj_item(di, wq, gc, sink):
                """One gate-chunk of one w-quarter: 6 MMs (N=512) + bias add."""
                wih_sb = wih_f if di == 0 else wih_b
                ppj = psB.tile([128, BC, 64], f32)   # 1 bank (512 f32)
                for dc in range(ND):
                    nc.tensor.matmul(
                        out=ppj,
                        lhsT=wih_sb[:, dc, gc * 128:(gc + 1) * 128],
                        rhs=pooledT[:, :, dc, wq * 64:(wq + 1) * 64],
                        start=(dc == 0), stop=(dc == ND - 1))
                bcol = bias_sb[:, di * NG + gc: di * NG + gc + 1]
                # pre is (w, gc, b)-ordered; psum is (b, w)
                dst = pre[:, di, wq * 64:(wq + 1) * 64, gc, :]
                src_ap = ppj.rearrange("p b w -> p w b")
                if sink == 0:
                    nc.vector.tensor_scalar(dst, src_ap, bcol, None, ALU.add)
                else:
                    nc.scalar.activation(dst, src_ap, AF.Identity,
                                         bias=bcol, scale=1.0)

            # ---- Phase C: the LSTM scan ----
            nc.vector.memset(hh[:, 0, :, :, 0], 0.0)     # fwd h_{-1} = 0
            nc.vector.memset(hh[:, 1, :, :, W], 0.0)     # bwd h_{W} = 0
            nc.vector.memset(cc, 0.0)

            def scan_mm(t, di):
                tf = t if di == 0 else W - 1 - t
                rslot = tf if di == 0 else tf + 1
                wslot = tf + 1 if di == 0 else tf
                # one bank for all 8 gate chunks; pre injected first (no h
                # dependency, runs during the previous step's EW tail)
                ps = psC.tile([128, NG, BC], f32, tag="ps")
                nc.tensor.matmul(out=ps, lhsT=ident_pre,
                                 rhs=pre[:, di, tf, :, :],
                                 start=True, stop=False)
                # kt-outer: all k0 matmuls gated only on the h write of the
                # previous step finishing (single strided write)
                for kt in range(KT):
                    for gc in range(NG):
                        nc.tensor.matmul(
                            out=ps[:, gc, :],
                            lhsT=whh_sb[:, di, kt, gc * 128:(gc + 1) * 128],
                            rhs=hh[:, di, kt, :, rslot],
                            start=False,
                            stop=(kt == KT - 1 and gc == NG - 1))
                return (di, ps, wslot)

            def scan_ew(st):
                di, ps, wslot = st
                # ONE sigmoid over all four gates (g rows pre-scaled x2):
                # chunks 0-1=i, 2-3=f, 4-5=o, 6-7=sigma(2x) for g
                s = sp.tile([128, NG, BC], f32)
                nc.scalar.activation(s, ps, AF.Sigmoid)
                gg = gp.tile([128, KT, BC], f32)
                nc.vector.tensor_scalar(gg, s[:, 6:8, :], 2.0, 1.0,
                                        ALU.mult, ALU.subtract)  # tanh(g)
                tmp = tp.tile([128, KT, BC], f32)
                nc.gpsimd.tensor_mul(tmp, s[:, 0:2, :], gg)      # i*g
                nc.vector.tensor_mul(cc[:, di], s[:, 2:4, :], cc[:, di])
                nc.vector.tensor_add(cc[:, di], cc[:, di], tmp)
                th = thp.tile([128, KT, BC], f32)
                nc.scalar.activation(th, cc[:, di], AF.Tanh)
                nc.vector.tensor_mul(hh[:, di, :, :, wslot], s[:, 4:6, :], th)

            def emit_out(di, b, wc):
                odram = outf if di == 0 else outb
                base = 1 if di == 0 else 0
                pst = psD.tile([128, KT, 128], bf16)
                for kt in range(KT):
                    nc.tensor.transpose(
                        pst[:, kt, :],
                        hh[:, di, kt, b, base + wc * 128: base + (wc + 1) * 128],
                        ident)
                stage = stg.tile([128, KT * 128], f32)
                if (b + wc) % 2 == 0:
                    nc.scalar.copy(stage, pst)
                else:
                    nc.vector.tensor_copy(stage, pst)
                nc.sync.dma_start(
                    out=odram.ap()[b, wc * 128:(wc + 1) * 128, :],
                    in_=stage)

            # Anti-phase emission: bwd's elementwise is emitted alongside
            # fwd's matmul burst and vice versa. Projection items are dripped
            # one gate-chunk per few steps into the scan's PE idle windows.
            for gc in range(NG):
                proj_item(0, 0, gc, gc % 2)
                proj_item(1, 3, gc, (gc + 1) % 2)
            # drip order: each quarter finishes well before its block starts
            drip = []
            for q, (qf, qb) in enumerate(((1, 2), (2, 1), (3, 0))):
                for gc in range(NG):
                    drip.append((0, qf, gc))
                    drip.append((1, qb, gc))
            emits = [(0, b, 0) for b in range(BC)] + [(1, b, 1) for b in range(BC)]

            pend_b = None
            di_sink = 0
            for t in range(W):
                if t == 192:
                    pb_ctx.close()
                    psD = bc_ctx.enter_context(
                        tc.tile_pool(name="psD", bufs=2, space="PSUM"))
                    stg = bc_ctx.enter_context(tc.tile_pool(name="stg", bufs=4))
                st_f = scan_mm(t, 0)
                if pend_b is not None:
                    scan_ew(pend_b)
                # drip one proj item every 4 steps (48 items / 192 steps)
                if t < 192 and t % 4 == 0:
                    di, wq, gc = drip[t // 4]
                    proj_item(di, wq, gc, di_sink)
                    di_sink ^= 1
                scan_ew(st_f)
                pend_b = scan_mm(t, 1)
                if t >= 192 and t % 4 == 0:
                    di, b, wc = emits[(t - 192) // 4]
                    emit_out(di, b, wc)
            scan_ew(pend_b)

            # ---- Phase D (part 2): remaining output chunks ----
            for b in range(BC):
                emit_out(0, b, 1)
                emit_out(1, b, 0)

    nc.compile()
    return nc


def get_nc():
    if "nc" not in _NC_CACHE:
        _NC_CACHE["nc"] = build_nc()
    return _NC_CACHE["nc"]


def prep_inputs(hidden_states, w_ih_f, w_hh_f, b_f, w_ih_b, w_hh_b, b_b,
                word_ids):
    """Host-side layout/dtype prep. Returns per-core input maps."""
    bf16 = ml_dtypes.bfloat16
    hidden_states = np.ascontiguousarray(hidden_states, dtype=np.float32)
    word_ids = np.asarray(word_ids)

    # scaled one-hot from the (index-only) word_ids
    M = (word_ids[:, :, None] == np.arange(W, dtype=word_ids.dtype)[None, None, :])
    M = M.astype(np.float32)
    counts = M.sum(axis=1)
    M *= (1.0 / np.maximum(counts, 1.0))[:, None, :]

    # gate permutation [i, f, g, o] -> [i, f, o, g]
    perm = np.concatenate([np.arange(0, 512), np.arange(768, 1024),
                           np.arange(512, 768)])

    def prep_dir(w_ih, w_hh, b):
        w_ih = np.asarray(w_ih, dtype=np.float32)[:, perm]
        w_hh = np.asarray(w_hh, dtype=np.float32)[:, perm]
        b = np.asarray(b, dtype=np.float32)[perm]
        # g-gate columns x2 so tanh(x) = 2*sigmoid(2x) - 1 works off the
        # single sigmoid table
        w_ih[:, 768:] *= 2.0
        w_hh[:, 768:] *= 2.0
        b[768:] *= 2.0
        return (w_ih.reshape(ND, 128, G),
                w_hh.reshape(KT, 128, G).astype(bf16),
                b.reshape(NG, 128))

    wf, whf, bf_ = prep_dir(w_ih_f, w_hh_f, b_f)
    wb, whb, bb_ = prep_dir(w_ih_b, w_hh_b, b_b)
    wih_all = np.ascontiguousarray(np.stack([wf, wb]))
    whh_all = np.ascontiguousarray(np.stack([whf, whb]))
    bias_all = np.ascontiguousarray(np.concatenate([bf_, bb_], axis=0))

    in_maps = []
    for c in range(NCORES):
        sl = slice(c * BC, (c + 1) * BC)
        in_maps.append({
            "hs": np.ascontiguousarray(
                hidden_states[sl].reshape(BC, NT, 128, D)),
            "msc": np.ascontiguousarray(M[sl].reshape(BC, NT, 128, W)),
            "wih": wih_all,
            "whh": whh_all,
            "bias": bias_all,
        })
    return in_maps


def assemble_output(results):
    out = np.empty((NCORES * BC, W, 2 * H), dtype=np.float32)
    for c, r in enumerate(results):
        sl = slice(c * BC, (c + 1) * BC)
        out[sl, :, :H] = r["outf"]
        out[sl, :, H:] = r["outb"]
    return out


def kernel(hidden_states, w_ih_f, w_hh_f, b_f, w_ih_b, w_hh_b, b_b,
           word_ids, max_seq_len=None, **_unused):
    from concourse.bass_utils import run_bass_kernel_spmd

    in_maps = prep_inputs(hidden_states, w_ih_f, w_hh_f, b_f,
                          w_ih_b, w_hh_b, b_b, word_ids)
    nc = get_nc()
    res = run_bass_kernel_spmd(nc, in_maps, list(range(NCORES)))
    _NC_CACHE["last_exec_time_ns"] = res.exec_time_ns
    return assemble_output(res.results)
